# revision 2
# baseline (speedup 1.0000x reference)
"""Correlation (FlowNet-style, max_displacement=4) on 8 TRN2 NeuronCores.

Full inputs x1, x2: [B=8, C=64, H=192, W=192] fp32. Output: [8, 81, 192, 192] fp32.
out[b, di*9+dj, h, w] = mean_c x1[b,c,h,w] * x2pad[b,c,h+di,w+dj]   (di,dj in [0,9))

Strategy: batch-parallel (1 batch per core). Per core the correlation is computed
as a banded Gram matrix on the TensorEngine: for each TH x TW (h,w) output tile,
one bf16 matmul with lhsT = x1 tile [K=64 channels, M=128 pixels] and rhs = padded
x2 window [64, NH*NW] produces all 81 displacement dot products of every tile
pixel inside a skewed band of the 128xN PSUM result. PSUM is evicted
(fp32->bf16) to SBUF by DVE/ACT in two-tile ops, and only the band parallelogram
(per-dh-group rectangles, (TW+8)/9 x the useful data) is DMA'd out.
The band is deskewed on the host with a zero-copy strided view. x1 is pre-scaled
by 1/64 on the host (exact, power of two) so the matmul output is directly the
channel mean.

TH=16/TW=8 (vs 8/16): same matmul shape (N=384) but the shipped band ratio drops
from 24/9=2.67x to 16/9=1.78x, cutting output HBM traffic by a third.

The h axis is split into two halves living on partitions 0-63 / 64-127, which
keeps DMA at full 128-partition width and lets the paired matmuls run
concurrently on disjoint PE row-groups (K=64 each). Inputs are loaded in three
h-chunks (separate tiles) interleaved with compute so the PE starts early.
"""

import sys
import types

import numpy as np
import ml_dtypes

import concourse.bacc as bacc
from concourse import mybir
from concourse.tile import TileContext
from concourse.bass_utils import run_bass_kernel_spmd

B, C, H, W = 8, 64, 192, 192
MAXD = 4
D = 2 * MAXD + 1  # 9
HP, WP = H + 2 * MAXD, W + 2 * MAXD  # 200, 200

TH, TW = 16, 8            # output tile (h, w) -> M = 128
NH, NW = TH + 2 * MAXD, TW + 2 * MAXD  # x2 window 24 x 16 -> N = 384
NSP = H // (2 * TH)       # 6 strips per partition-half
N_WT = W // TW            # 24 w-tiles
HHALF = H // 2            # 96 rows per partition-half
SLAB = HHALF + 2 * MAXD   # 104 padded x2 rows per half
BCOL = D * NW             # 144 band columns per dh-group

# Input h-chunking: strip ranges per chunk and the x2 slab rows they need.
X1_CHUNKS = [(0, 2), (2, 4), (4, 6)]              # strip ranges
X2_CHUNKS = [(0, 40), (32, 72), (64, 104)]        # x2 local row ranges

BF16 = ml_dtypes.bfloat16


def _install_axon_trace_shim():
    """The image's antenv package lacks axon_hooks; run_bass_kernel_spmd
    crashes on import when trace=True. Provide the hook from the boot module
    so tracing works instead of raising."""
    if "antenv.axon_hooks" in sys.modules:
        return
    try:
        import trn_agent_boot.trn_boot as tb

        hook = tb._ntff_profile_via_ctypes("/opt/axon/libaxon_pjrt.so")
    except Exception:
        hook = None
    mod = types.ModuleType("antenv.axon_hooks")
    mod.get_axon_ntff_profile_hook = lambda: hook
    mod.set_axon_ntff_profile_hook = lambda h: None
    sys.modules["antenv.axon_hooks"] = mod


def build_nc():
    nc = bacc.Bacc("TRN2", target_bir_lowering=False, debug=False)
    # x1 arrives pre-tiled: [128, strip, wtile, 128 pixels] — walrus requires
    # the matmul weights AP to have a single free dimension.
    x1s = nc.dram_tensor("x1s", [128, NSP, N_WT, TH * TW], mybir.dt.bfloat16, kind="ExternalInput")
    x2s = nc.dram_tensor("x2s", [128, SLAB, WP], mybir.dt.bfloat16, kind="ExternalInput")
    y = nc.dram_tensor("y", [NSP, 2, TH, TW, BCOL, N_WT], mybir.dt.bfloat16, kind="ExternalOutput")

    with TileContext(nc) as tc:
        with (
            tc.tile_pool(name="imgs", bufs=1) as imgs,
            tc.tile_pool(name="outs", bufs=2) as outs,
            tc.tile_pool(name="psum", bufs=4, space="PSUM") as psum,
        ):
            # Chunked input tiles (separate tiles -> precise chunk->matmul deps).
            x1c, x2c = [], []
            for ci in range(3):
                s0, s1 = X1_CHUNKS[ci]
                r0, r1 = X2_CHUNKS[ci]
                x2t = imgs.tile([128, r1 - r0, WP], mybir.dt.bfloat16, tag=f"x2c{ci}")
                nc.sync.dma_start(out=x2t[:], in_=x2s[:, r0:r1, :])
                x1t = imgs.tile([128, s1 - s0, N_WT, TH * TW], mybir.dt.bfloat16, tag=f"x1c{ci}")
                nc.scalar.dma_start(out=x1t[:], in_=x1s[:, s0:s1])
                x2c.append(x2t)
                x1c.append(x1t)

            copy_k = 0
            for sp in range(NSP):
                ci = next(i for i, (s0, s1) in enumerate(X1_CHUNKS) if s0 <= sp < s1)
                hl = sp * TH - X2_CHUNKS[ci][0]   # row offset within x2 chunk
                spl = sp - X1_CHUNKS[ci][0]       # strip offset within x1 chunk
                # ybuf is column-major over w-tiles ([col, t]) so each band
                # rectangle is one contiguous run per partition.
                ybufs = [outs.tile([128, NH * NW, N_WT], mybir.dt.bfloat16,
                                   name=f"ybuf{half}_{sp}", tag=f"ybuf{half}")
                         for half in range(2)]
                for tp in range(N_WT // 2):       # pairs of w-tiles
                    # Interleave the two partition halves so adjacent matmuls
                    # sit on disjoint PE row-groups and execute concurrently.
                    for half in range(2):
                        p0 = 64 * half
                        pt = psum.tile([128, 1024], mybir.dt.float32)
                        for u in range(2):
                            t = 2 * tp + u
                            w0 = t * TW
                            nc.tensor.matmul(
                                pt[:, 512 * u:512 * u + NH * NW],
                                lhsT=x1c[ci][p0:p0 + 64, spl, t, :],
                                rhs=x2c[ci][p0:p0 + 64, hl:hl + NH, w0:w0 + NW],
                                start=True, stop=True,
                            )
                        # Evict both tiles with one op; alternate DVE / ACT.
                        src = pt[:].rearrange("p (a b) -> p b a", a=2)[:, 0:NH * NW, :]
                        dst = ybufs[half][:, :, 2 * tp:2 * tp + 2]
                        if copy_k % 2 == 0:
                            nc.vector.tensor_copy(dst, src)
                        else:
                            nc.scalar.copy(dst, src)
                        copy_k += 1
                # Band parallelogram out: per dh-group g, columns
                # [NW*g, NW*g+BCOL) of partitions [TW*g, TW*g+TW) hold all
                # (di, dj) results for those rows — one contiguous run per
                # partition.
                for half in range(2):
                    for g in range(TH):
                        eng = nc.sync if (half + g) % 2 == 0 else nc.scalar
                        eng.dma_start(
                            out=y[sp, half, g],
                            in_=ybufs[half][TW * g:TW * g + TW, NW * g:NW * g + BCOL, :],
                        )

    nc.compile()
    return nc


_NC_CACHE = None


def _get_nc():
    global _NC_CACHE
    if _NC_CACHE is None:
        _NC_CACHE = build_nc()
    return _NC_CACHE


def _prep_inputs(x1, x2):
    """Host-side shard prep: scale, pad, split h into partition halves, bf16."""
    in_maps = []
    x1 = np.asarray(x1, dtype=np.float32)
    x2 = np.asarray(x2, dtype=np.float32)
    x1h = (x1 * (1.0 / C)).astype(BF16)
    x2h = x2.astype(BF16)
    for b in range(B):
        # x1: [64, 192, 192] -> pre-tiled [128 = half*64+c, sp, t, dh*TW+dw]
        a = x1h[b].reshape(C, 2, NSP, TH, N_WT, TW)
        a = a.transpose(1, 0, 2, 4, 3, 5).reshape(128, NSP, N_WT, TH * TW)
        # x2: pad to [64, 200, 200], two overlapping 104-row slabs
        p = np.zeros((C, HP, WP), dtype=BF16)
        p[:, MAXD:MAXD + H, MAXD:MAXD + W] = x2h[b]
        s = np.stack([p[:, 0:SLAB, :], p[:, HHALF:HHALF + SLAB, :]], axis=0)
        s = s.reshape(2 * C, SLAB, WP)
        in_maps.append({"x1s": np.ascontiguousarray(a), "x2s": np.ascontiguousarray(s)})
    return in_maps


def _deskew(yb):
    """yb: [NSP, 2, TH, TW, BCOL, N_WT] fp32 (one batch) -> [81, 192, 192]."""
    s_sp, s_half, s_g, s_dw, s_c, s_t = yb.strides
    v = np.lib.stride_tricks.as_strided(
        yb,
        shape=(D, D, 2, NSP, TH, N_WT, TW),
        strides=(NW * s_c, s_c, s_half, s_sp, s_g, s_t, s_dw + s_c),
    )
    return np.ascontiguousarray(v).reshape(D * D, H, W)


def kernel(x1, x2):
    _install_axon_trace_shim()
    nc = _get_nc()
    in_maps = _prep_inputs(x1, x2)
    res = run_bass_kernel_spmd(nc, in_maps, core_ids=list(range(B)))
    kernel.last_results = res
    out = np.empty((B, D * D, H, W), dtype=np.float32)
    for b in range(B):
        yb = np.asarray(res.results[b]["y"]).astype(np.float32)
        out[b] = _deskew(yb)
    return out


# revision 3
# speedup vs baseline: 1.0703x; 1.0703x over previous
"""Correlation (FlowNet-style, max_displacement=4) on 8 TRN2 NeuronCores.

Full inputs x1, x2: [B=8, C=64, H=192, W=192] fp32. Output: [8, 81, 192, 192] fp32.
out[b, di*9+dj, h, w] = mean_c x1[b,c,h,w] * x2pad[b,c,h+di,w+dj]   (di,dj in [0,9))

Strategy: batch-parallel (1 batch per core). Per core the correlation is computed
as a banded Gram matrix on the TensorEngine: for each TH x TW (h,w) output tile,
one bf16 matmul with lhsT = x1 tile [K=64 channels, M=128 pixels] and rhs = padded
x2 window [64, NH*NW] produces all 81 displacement dot products of every tile
pixel inside a skewed band of the 128xN PSUM result. PSUM is evicted
(fp32->bf16) to SBUF by DVE/ACT in two-tile ops, and only the band parallelogram
(per-dh-group rectangles, (TW+8)/9 x the useful data) is DMA'd out.
The band is deskewed on the host with a zero-copy strided view. x1 is pre-scaled
by 1/64 on the host (exact, power of two) so the matmul output is directly the
channel mean.

TH=16/TW=8: same matmul shape (N=384) as 8/16 but the shipped band ratio drops
from 24/9=2.67x to 16/9=1.78x, cutting output HBM traffic by a third.

Both partition-halves' bands live in ONE ybuf tile ([128, half, col, wtile]) so
each per-g out-DMA ships both halves: 16 descriptors per DMA (2 per partition),
which sprays across all 16 SDMA engines (8-descriptor DMAs only ever touch
engines 0-7), and 96 DMA instructions instead of 192. DMA issue alternates
between the two HWDGE rings (sync / scalar).

The h axis is split into two halves living on partitions 0-63 / 64-127, which
keeps DMA at full 128-partition width and lets the paired matmuls run
concurrently on disjoint PE row-groups (K=64 each). Inputs are loaded in three
h-chunks (separate tiles) interleaved with compute so the PE starts early.
"""

import sys
import types

import numpy as np
import ml_dtypes

import concourse.bacc as bacc
from concourse import mybir
from concourse.tile import TileContext
from concourse.bass_utils import run_bass_kernel_spmd

B, C, H, W = 8, 64, 192, 192
MAXD = 4
D = 2 * MAXD + 1  # 9
HP, WP = H + 2 * MAXD, W + 2 * MAXD  # 200, 200

TH, TW = 16, 8            # output tile (h, w) -> M = 128
NH, NW = TH + 2 * MAXD, TW + 2 * MAXD  # x2 window 24 x 16 -> N = 384
NSP = H // (2 * TH)       # 6 strips per partition-half
N_WT = W // TW            # 24 w-tiles
HHALF = H // 2            # 96 rows per partition-half
SLAB = HHALF + 2 * MAXD   # 104 padded x2 rows per half
BCOL = D * NW             # 144 band columns per dh-group

# Input h-chunking: strip ranges per chunk and the x2 slab rows they need.
X1_CHUNKS = [(0, 2), (2, 4), (4, 6)]              # strip ranges
X2_CHUNKS = [(0, 40), (32, 72), (64, 104)]        # x2 local row ranges

BF16 = ml_dtypes.bfloat16


def _install_axon_trace_shim():
    """The image's antenv package lacks axon_hooks; run_bass_kernel_spmd
    crashes on import when trace=True. Provide the hook from the boot module
    so tracing works instead of raising."""
    if "antenv.axon_hooks" in sys.modules:
        return
    try:
        import trn_agent_boot.trn_boot as tb

        hook = tb._ntff_profile_via_ctypes("/opt/axon/libaxon_pjrt.so")
    except Exception:
        hook = None
    mod = types.ModuleType("antenv.axon_hooks")
    mod.get_axon_ntff_profile_hook = lambda: hook
    mod.set_axon_ntff_profile_hook = lambda h: None
    sys.modules["antenv.axon_hooks"] = mod


def build_nc():
    nc = bacc.Bacc("TRN2", target_bir_lowering=False, debug=False)
    # x1 arrives pre-tiled: [128, strip, wtile, 128 pixels] — walrus requires
    # the matmul weights AP to have a single free dimension.
    x1s = nc.dram_tensor("x1s", [128, NSP, N_WT, TH * TW], mybir.dt.bfloat16, kind="ExternalInput")
    x2s = nc.dram_tensor("x2s", [128, SLAB, WP], mybir.dt.bfloat16, kind="ExternalInput")
    y = nc.dram_tensor("y", [NSP, TH, TW, 2, BCOL, N_WT], mybir.dt.bfloat16, kind="ExternalOutput")

    with TileContext(nc) as tc:
        with (
            tc.tile_pool(name="imgs", bufs=1) as imgs,
            tc.tile_pool(name="outs", bufs=2) as outs,
            tc.tile_pool(name="psum", bufs=4, space="PSUM") as psum,
        ):
            # Chunked input tiles (separate tiles -> precise chunk->matmul deps).
            x1c, x2c = [], []
            for ci in range(3):
                s0, s1 = X1_CHUNKS[ci]
                r0, r1 = X2_CHUNKS[ci]
                x2t = imgs.tile([128, r1 - r0, WP], mybir.dt.bfloat16, tag=f"x2c{ci}")
                nc.sync.dma_start(out=x2t[:], in_=x2s[:, r0:r1, :])
                x1t = imgs.tile([128, s1 - s0, N_WT, TH * TW], mybir.dt.bfloat16, tag=f"x1c{ci}")
                nc.scalar.dma_start(out=x1t[:], in_=x1s[:, s0:s1])
                x2c.append(x2t)
                x1c.append(x1t)

            copy_k = 0
            for sp in range(NSP):
                ci = next(i for i, (s0, s1) in enumerate(X1_CHUNKS) if s0 <= sp < s1)
                hl = sp * TH - X2_CHUNKS[ci][0]   # row offset within x2 chunk
                spl = sp - X1_CHUNKS[ci][0]       # strip offset within x1 chunk
                # Both halves in one tile; [col, t] minor so each band
                # rectangle is one contiguous run per (partition, half).
                ybuf = outs.tile([128, 2, NH * NW, N_WT], mybir.dt.bfloat16,
                                 name=f"ybuf_{sp}", tag="ybuf")
                for tp in range(N_WT // 2):       # pairs of w-tiles
                    # Interleave the two partition halves so adjacent matmuls
                    # sit on disjoint PE row-groups and execute concurrently.
                    for half in range(2):
                        p0 = 64 * half
                        pt = psum.tile([128, 1024], mybir.dt.float32)
                        for u in range(2):
                            t = 2 * tp + u
                            w0 = t * TW
                            nc.tensor.matmul(
                                pt[:, 512 * u:512 * u + NH * NW],
                                lhsT=x1c[ci][p0:p0 + 64, spl, t, :],
                                rhs=x2c[ci][p0:p0 + 64, hl:hl + NH, w0:w0 + NW],
                                start=True, stop=True,
                            )
                        # Evict both tiles with one op; alternate DVE / ACT.
                        src = pt[:].rearrange("p (a b) -> p b a", a=2)[:, 0:NH * NW, :]
                        dst = ybuf[:, half, :, 2 * tp:2 * tp + 2]
                        if copy_k % 2 == 0:
                            nc.vector.tensor_copy(dst, src)
                        else:
                            nc.scalar.copy(dst, src)
                        copy_k += 1
                # Band parallelogram out: per dh-group g, columns
                # [NW*g, NW*g+BCOL) of partitions [TW*g, TW*g+TW), both halves
                # at once: 16 descriptors -> sprayed over all 16 SDMA engines.
                for g in range(TH):
                    eng = nc.sync if g % 2 == 0 else nc.scalar
                    eng.dma_start(
                        out=y[sp, g],
                        in_=ybuf[TW * g:TW * g + TW, :, NW * g:NW * g + BCOL, :],
                    )

    nc.compile()
    return nc


_NC_CACHE = None


def _get_nc():
    global _NC_CACHE
    if _NC_CACHE is None:
        _NC_CACHE = build_nc()
    return _NC_CACHE


def _prep_inputs(x1, x2):
    """Host-side shard prep: scale, pad, split h into partition halves, bf16."""
    in_maps = []
    x1 = np.asarray(x1, dtype=np.float32)
    x2 = np.asarray(x2, dtype=np.float32)
    x1h = (x1 * (1.0 / C)).astype(BF16)
    x2h = x2.astype(BF16)
    for b in range(B):
        # x1: [64, 192, 192] -> pre-tiled [128 = half*64+c, sp, t, dh*TW+dw]
        a = x1h[b].reshape(C, 2, NSP, TH, N_WT, TW)
        a = a.transpose(1, 0, 2, 4, 3, 5).reshape(128, NSP, N_WT, TH * TW)
        # x2: pad to [64, 200, 200], two overlapping 104-row slabs
        p = np.zeros((C, HP, WP), dtype=BF16)
        p[:, MAXD:MAXD + H, MAXD:MAXD + W] = x2h[b]
        s = np.stack([p[:, 0:SLAB, :], p[:, HHALF:HHALF + SLAB, :]], axis=0)
        s = s.reshape(2 * C, SLAB, WP)
        in_maps.append({"x1s": np.ascontiguousarray(a), "x2s": np.ascontiguousarray(s)})
    return in_maps


def _deskew(yb):
    """yb: [NSP, TH, TW, 2, BCOL, N_WT] fp32 (one batch) -> [81, 192, 192]."""
    s_sp, s_g, s_dw, s_half, s_c, s_t = yb.strides
    v = np.lib.stride_tricks.as_strided(
        yb,
        shape=(D, D, 2, NSP, TH, N_WT, TW),
        strides=(NW * s_c, s_c, s_half, s_sp, s_g, s_t, s_dw + s_c),
    )
    return np.ascontiguousarray(v).reshape(D * D, H, W)


def kernel(x1, x2):
    _install_axon_trace_shim()
    nc = _get_nc()
    in_maps = _prep_inputs(x1, x2)
    res = run_bass_kernel_spmd(nc, in_maps, core_ids=list(range(B)))
    kernel.last_results = res
    out = np.empty((B, D * D, H, W), dtype=np.float32)
    for b in range(B):
        yb = np.asarray(res.results[b]["y"]).astype(np.float32)
        out[b] = _deskew(yb)
    return out


# revision 4
# speedup vs baseline: 1.4565x; 1.3608x over previous
"""Correlation (FlowNet-style, max_displacement=4) on 8 TRN2 NeuronCores.

Full inputs x1, x2: [B=8, C=64, H=192, W=192] fp32. Output: [8, 81, 192, 192] fp32.
out[b, di*9+dj, h, w] = mean_c x1[b,c,h,w] * x2pad[b,c,h+di,w+dj]   (di,dj in [0,9))

Strategy: batch-parallel (1 batch per core). Per core the correlation is computed
as a banded Gram matrix on the TensorEngine: for each 16x8 (h,w) output tile,
one bf16 matmul with lhsT = x1 tile [K=64 channels, M=128 pixels] and rhs = padded
x2 window [64, 24*16=384] produces all 81 displacement dot products of every tile
pixel inside a skewed band of the 128x384 PSUM result. PSUM is evicted
(fp32->bf16) to SBUF by DVE/ACT in two-tile ops. The band is shipped in
4-dh-group blocks: partitions [32b, 32b+32) x band-column union [64b, 64b+192)
x both halves — 64 descriptors of 9216 B per DMA, which sprays across all 16
SDMA engines (8/16-descriptor DMAs only ever land on engines 0-7) at good
per-descriptor efficiency, for 1.33x byte inflation over the exact band
parallelogram (2.37x the useful output vs 4.74x if the whole PSUM band were
shipped). Only 4 out-DMAs per strip (24 total), alternating between the two
HWDGE rings (sync/scalar), so sequencer descriptor-gen (~640+40*ndesc ns per
DMA) stays off the critical path. The band is deskewed on the host with a
zero-copy strided view. x1 is pre-scaled by 1/64 on the host (exact) so the
matmul output is directly the channel mean.

The h axis is split into two halves living on partitions 0-63 / 64-127 (K=64
each), interleaved so paired matmuls run concurrently on disjoint PE
row-groups. Inputs are loaded in three h-chunks (separate tiles, small first
chunk) interleaved with compute so the PE starts early.
"""

import sys
import types

import numpy as np
import ml_dtypes

import concourse.bacc as bacc
from concourse import mybir
from concourse.tile import TileContext
from concourse.bass_utils import run_bass_kernel_spmd

B, C, H, W = 8, 64, 192, 192
MAXD = 4
D = 2 * MAXD + 1  # 9
HP, WP = H + 2 * MAXD, W + 2 * MAXD  # 200, 200

TH, TW = 16, 8            # output tile (h, w) -> M = 128
NH, NW = TH + 2 * MAXD, TW + 2 * MAXD  # x2 window 24 x 16 -> N = 384
NSP = H // (2 * TH)       # 6 strips per partition-half
N_WT = W // TW            # 24 w-tiles
HHALF = H // 2            # 96 rows per partition-half
SLAB = HHALF + 2 * MAXD   # 104 padded x2 rows per half
BCOL = D * NW             # 144 band columns per dh-group
GB = 4                    # dh-groups per out-DMA block
NB = TH // GB             # 4 blocks per strip
BW = BCOL + (GB - 1) * NW  # 192 block band columns

# Input h-chunking: strip ranges per chunk and the x2 slab rows they need.
X1_CHUNKS = [(0, 1), (1, 3), (3, 6)]              # strip ranges
X2_CHUNKS = [(0, 24), (16, 56), (40, 104)]        # x2 local row ranges

BF16 = ml_dtypes.bfloat16


def _install_axon_trace_shim():
    """The image's antenv package lacks axon_hooks; run_bass_kernel_spmd
    crashes on import when trace=True. Provide the hook from the boot module
    so tracing works instead of raising."""
    if "antenv.axon_hooks" in sys.modules:
        return
    try:
        import trn_agent_boot.trn_boot as tb

        hook = tb._ntff_profile_via_ctypes("/opt/axon/libaxon_pjrt.so")
    except Exception:
        hook = None
    mod = types.ModuleType("antenv.axon_hooks")
    mod.get_axon_ntff_profile_hook = lambda: hook
    mod.set_axon_ntff_profile_hook = lambda h: None
    sys.modules["antenv.axon_hooks"] = mod


def build_nc():
    nc = bacc.Bacc("TRN2", target_bir_lowering=False, debug=False)
    # x1 arrives pre-tiled: [128, strip, wtile, 128 pixels] — walrus requires
    # the matmul weights AP to have a single free dimension.
    x1s = nc.dram_tensor("x1s", [128, NSP, N_WT, TH * TW], mybir.dt.bfloat16, kind="ExternalInput")
    x2s = nc.dram_tensor("x2s", [128, SLAB, WP], mybir.dt.bfloat16, kind="ExternalInput")
    y = nc.dram_tensor("y", [NSP, NB, GB * TW, 2, BW, N_WT], mybir.dt.bfloat16, kind="ExternalOutput")

    with TileContext(nc) as tc:
        with (
            tc.tile_pool(name="imgs", bufs=1) as imgs,
            tc.tile_pool(name="outs", bufs=2) as outs,
            tc.tile_pool(name="psum", bufs=4, space="PSUM") as psum,
        ):
            # Chunked input tiles (separate tiles -> precise chunk->matmul deps).
            x1c, x2c = [], []
            for ci in range(3):
                s0, s1 = X1_CHUNKS[ci]
                r0, r1 = X2_CHUNKS[ci]
                x2t = imgs.tile([128, r1 - r0, WP], mybir.dt.bfloat16, tag=f"x2c{ci}")
                nc.sync.dma_start(out=x2t[:], in_=x2s[:, r0:r1, :])
                x1t = imgs.tile([128, s1 - s0, N_WT, TH * TW], mybir.dt.bfloat16, tag=f"x1c{ci}")
                nc.scalar.dma_start(out=x1t[:], in_=x1s[:, s0:s1])
                x2c.append(x2t)
                x1c.append(x1t)

            copy_k = 0
            for sp in range(NSP):
                ci = next(i for i, (s0, s1) in enumerate(X1_CHUNKS) if s0 <= sp < s1)
                hl = sp * TH - X2_CHUNKS[ci][0]   # row offset within x2 chunk
                spl = sp - X1_CHUNKS[ci][0]       # strip offset within x1 chunk
                # Both halves in one tile; [col, t] minor so each band block
                # is one contiguous run per (partition, half).
                ybuf = outs.tile([128, 2, NH * NW, N_WT], mybir.dt.bfloat16,
                                 name=f"ybuf_{sp}", tag="ybuf")
                for tp in range(N_WT // 2):       # pairs of w-tiles
                    # Interleave the two partition halves so adjacent matmuls
                    # sit on disjoint PE row-groups and execute concurrently.
                    for half in range(2):
                        p0 = 64 * half
                        pt = psum.tile([128, 1024], mybir.dt.float32)
                        for u in range(2):
                            t = 2 * tp + u
                            w0 = t * TW
                            nc.tensor.matmul(
                                pt[:, 512 * u:512 * u + NH * NW],
                                lhsT=x1c[ci][p0:p0 + 64, spl, t, :],
                                rhs=x2c[ci][p0:p0 + 64, hl:hl + NH, w0:w0 + NW],
                                start=True, stop=True,
                            )
                        # Evict both tiles with one op; alternate DVE / ACT.
                        src = pt[:].rearrange("p (a b) -> p b a", a=2)[:, 0:NH * NW, :]
                        dst = ybuf[:, half, :, 2 * tp:2 * tp + 2]
                        if copy_k % 2 == 0:
                            nc.vector.tensor_copy(dst, src)
                        else:
                            nc.scalar.copy(dst, src)
                        copy_k += 1
                # Band out in 4-group blocks: partitions [32b, 32b+32), column
                # union [64b, 64b+192), both halves: 64 descriptors x 9216 B.
                for blk in range(NB):
                    eng = nc.sync if (sp + blk) % 2 == 0 else nc.scalar
                    eng.dma_start(
                        out=y[sp, blk],
                        in_=ybuf[32 * blk:32 * blk + 32, :,
                                 NW * GB * blk:NW * GB * blk + BW, :],
                    )

    nc.compile()
    return nc


_NC_CACHE = None


def _get_nc():
    global _NC_CACHE
    if _NC_CACHE is None:
        _NC_CACHE = build_nc()
    return _NC_CACHE


def _prep_inputs(x1, x2):
    """Host-side shard prep: scale, pad, split h into partition halves, bf16."""
    in_maps = []
    x1 = np.asarray(x1, dtype=np.float32)
    x2 = np.asarray(x2, dtype=np.float32)
    x1h = (x1 * (1.0 / C)).astype(BF16)
    x2h = x2.astype(BF16)
    for b in range(B):
        # x1: [64, 192, 192] -> pre-tiled [128 = half*64+c, sp, t, dh*TW+dw]
        a = x1h[b].reshape(C, 2, NSP, TH, N_WT, TW)
        a = a.transpose(1, 0, 2, 4, 3, 5).reshape(128, NSP, N_WT, TH * TW)
        # x2: pad to [64, 200, 200], two overlapping 104-row slabs
        p = np.zeros((C, HP, WP), dtype=BF16)
        p[:, MAXD:MAXD + H, MAXD:MAXD + W] = x2h[b]
        s = np.stack([p[:, 0:SLAB, :], p[:, HHALF:HHALF + SLAB, :]], axis=0)
        s = s.reshape(2 * C, SLAB, WP)
        in_maps.append({"x1s": np.ascontiguousarray(a), "x2s": np.ascontiguousarray(s)})
    return in_maps


def _deskew(yb):
    """yb: [NSP, NB, GB*TW, 2, BW, N_WT] fp32 (one batch) -> [81, 192, 192].

    h = half*96 + sp*TH + 4*blk + gin,  w = t*TW + dw; the value for
    displacement (di, dj) at (gin, dw) sits at block column
    16*gin + 16*di + dw + dj of partition gin*8+dw.
    """
    s_sp, s_blk, s_p, s_half, s_c, s_t = yb.strides
    v = np.lib.stride_tricks.as_strided(
        yb,
        shape=(D, D, 2, NSP, NB, GB, N_WT, TW),
        strides=(NW * s_c, s_c, s_half, s_sp, s_blk,
                 TW * s_p + NW * s_c, s_t, s_p + s_c),
    )
    return np.ascontiguousarray(v).reshape(D * D, H, W)


def kernel(x1, x2):
    _install_axon_trace_shim()
    nc = _get_nc()
    in_maps = _prep_inputs(x1, x2)
    res = run_bass_kernel_spmd(nc, in_maps, core_ids=list(range(B)))
    kernel.last_results = res
    out = np.empty((B, D * D, H, W), dtype=np.float32)
    for b in range(B):
        yb = np.asarray(res.results[b]["y"]).astype(np.float32)
        out[b] = _deskew(yb)
    return out
